# revision 24
# baseline (speedup 1.0000x reference)
"""Trainium2 Bass kernel for nn_AttentionBlock (multi-head attention block).

Reference computation (fp32):
    q = einsum('bsi,hbik->hbsk', x, Mq)   # Mq: (H,1,I,K) broadcast over b
    k = einsum('bsi,hbik->hbsk', x, Mk)
    v = einsum('bsi,hbiv->hbsv', x, Mv)
    scores  = einsum('hbsk,hbtk->hbst', q, k) / sqrt(K)
    weights = softmax(scores, axis=-1)
    out     = einsum('hbst,hbtv->hbsv', weights, v)   # (H,B,S,V)

Sharding: 8 cores = 4 batches x 2 head-groups (4 heads each). Attention is
independent per (batch, head) so no cross-core communication is needed.

Per-core kernel design (one batch b, 4 heads):
  - xT = x.T via PE transposes in fp16 (x cast on DVE first)  [I on partitions]
  - QT/KT projections with two heads packed per matmul (lhsT = [Mq_h | Mq_h'],
    128 cols) -> QT/KT packs [128p, S] fp16, head h in partitions 0:64,
    head h' in 64:128.
  - V projection with all 4 heads packed on the moving side (rhs = [Mv_0..Mv_3],
    N=512) -> V natural [t, v] fp16 tiles, with a ones-column appended.
  - scores computed transposed (scoresT[t,s] = k_t . q_s / sqrt(K)) with the two
    heads of a pair issued to disjoint PE row-groups (tile_position) so the
    64-deep contractions run concurrently at full array utilization.
  - exp via ACT directly PSUM -> SBUF fp16 (scale=1/sqrt(K) folded in; softmax
    max-subtraction skipped: logits are O(1) for this problem so exp is safe).
  - AV: out[s, 0:128] and the softmax denominator in one accumulation:
    lhsT = expT chunk [t,128s], rhs = [V | ones] [t, 129]. Column 128 of the
    PSUM result is sum_t exp = denominator, per-partition.
  - evict: out = psum[:, 0:V] * (1/denom) via DVE, DMA to DRAM in natural
    [s, v] layout.
Host side: shard inputs, run SPMD on 8 cores, reassemble (H,B,S,V).
"""

import sys

sys.path.insert(0, "/opt/trn_rl_repo")

import math
from contextlib import ExitStack

import numpy as np

import concourse.bass as bass
import concourse.mybir as mybir
import concourse.tile as tile
from concourse import bacc
from concourse.masks import make_identity

F32 = mybir.dt.float32
F16 = mybir.dt.float16


def build_attention_nc(S=2048, I=1024, K=64, V=128, HPC=4, reps=1, tune=None):
    """Build the single-core Bass program (SPMD: same program on all cores).

    reps > 1 re-emits the whole computation (for timing calibration: the
    per-rep delta of one NEFF execution isolates device exec time from
    dispatch overhead).
    """
    assert S % 512 == 0 and I % 128 == 0 and V == 128 and K == 64
    assert HPC % 2 == 0
    NSG = S // 512  # s groups of 512 queries
    NST = S // 128  # 128-row tiles (both s and t)
    NCI = I // 128  # contraction chunks for projections
    NPAIR = HPC // 2
    SCALE = 1.0 / math.sqrt(K)

    nc = bacc.Bacc("TRN2", target_bir_lowering=False)
    x = nc.dram_tensor("x", [S, I], F32, kind="ExternalInput")
    mq = nc.dram_tensor("mq", [HPC, I, K], F32, kind="ExternalInput")
    mk = nc.dram_tensor("mk", [HPC, I, K], F32, kind="ExternalInput")
    mv = nc.dram_tensor("mv", [HPC, I, V], F32, kind="ExternalInput")
    out = nc.dram_tensor("out", [HPC, S, V], F32, kind="ExternalOutput")

    tune = dict(tune or {})
    with tile.TileContext(nc) as tc:
        for rep in range(reps):
            _emit_rep(nc, tc, rep, x, mq, mk, mv, out,
                      S, I, K, V, HPC, NSG, NST, NCI, NPAIR, SCALE, tune)
    nc.compile()
    return nc


def _emit_rep(nc, tc, rep, x, mq, mk, mv, out,
              S, I, K, V, HPC, NSG, NST, NCI, NPAIR, SCALE, tune):
    T = tune.get
    if T("act_evict", 0):
        def ev_copy(dst, src):
            nc.scalar.copy(dst, src)
    else:
        def ev_copy(dst, src):
            nc.vector.tensor_copy(dst, src)
    with ExitStack() as persist_ctx:
        persist = persist_ctx.enter_context(
            tc.tile_pool(name=f"persist{rep}", bufs=1)
        )

        # ---------------- persistent SBUF tensors ----------------
        # fp32 identity built on gpsimd, then cast to fp16 on DVE so that every
        # transpose-matmul dependency lives on the DVE semaphore (the S3_LW
        # self-loading matmul encoding only supports a single sync wait).
        ident32 = persist.tile([128, 128], F32, tag="ident32")
        make_identity(nc, ident32)
        ident = persist.tile([128, 128], F16, tag="ident")
        ev_copy(ident[:], ident32[:])

        xT = persist.tile([128, NCI, S], F16, tag="xT")  # x transposed
        qt = [persist.tile([128, S], F16, tag=f"qt{p}", name=f"qt{rep}_{p}") for p in range(NPAIR)]
        kt = [persist.tile([128, S], F16, tag=f"kt{p}", name=f"kt{rep}_{p}") for p in range(NPAIR)]
        # V per head: [t-part, chunk, V+1 (ones) padded]
        vsb = [persist.tile([128, NST, V + 4], F16, tag=f"v{h}", name=f"v{rep}_{h}") for h in range(HPC)]
        for h in range(HPC):
            nc.vector.memset(vsb[h][:, :, V : V + 1], 1.0)

        mqp = [persist.tile([128, NCI, 128], F16, tag=f"mqp{p}", name=f"mqp{rep}_{p}") for p in range(NPAIR)]
        mkp = [persist.tile([128, NCI, 128], F16, tag=f"mkp{p}", name=f"mkp{rep}_{p}") for p in range(NPAIR)]
        mvp = persist.tile([128, NCI, HPC * V], F16, tag="mvp")

        FUSE = T("fuse", 0)
        stage_ctx = ExitStack()   # SBUF staging; always closed after phase 2
        psum_ctx = ExitStack()    # projection-phase PSUM pools
        stage = stage_ctx.enter_context(tc.tile_pool(name=f"stage{rep}", bufs=1))
        xstage = stage_ctx.enter_context(
            tc.tile_pool(name=f"xstage{rep}", bufs=T("xstage", 3))
        )
        pproj = psum_ctx.enter_context(
            tc.tile_pool(name=f"pproj{rep}", bufs=T("pproj", 2), space="PSUM")
        )
        ptr_ctx = ExitStack()
        ptr = ptr_ctx.enter_context(
            tc.tile_pool(name=f"ptr{rep}", bufs=T("ptr", 1 if FUSE else 2), space="PSUM")
        )
        # under FUSE, projection psum tiles share one tag (2 banks total) and
        # the attention-phase pools are opened alongside so PSUM fits in 8
        # banks concurrently -> the scheduler can overlap pair-0 scores/exp
        # with the tail of the projection phase.
        ptag = (lambda s: "pp") if FUSE else (lambda s: s)
        if True:
            # ------------- phase 0: load + pack + cast weights -------------
            # All weights land in one fp32 staging buffer via disjoint-slice
            # DMAs (no slot reuse -> at most one sync wait per HWDGE DMA).
            WQ, WK, WV = 0, HPC * K, 2 * HPC * K
            wstack = stage.tile([128, NCI, 2 * HPC * K + HPC * V], F32, tag="wstack")
            xbig = stage.tile([128, NST, I], F32, tag="xbig")
            xr = x.rearrange("(st p) i -> p st i", p=128)
            XS = T("xsplit", 2)  # st-tiles per x DMA
            if T("dma_first", 0):
                nc.sync.dma_start(xbig[:, 0:XS, :], xr[:, 0:XS, :])
            for h in range(HPC):
                nc.sync.dma_start(
                    wstack[:, :, WQ + h * K : WQ + (h + 1) * K],
                    mq[h].rearrange("(c i) k -> i c k", i=128),
                )
                nc.sync.dma_start(
                    wstack[:, :, WK + h * K : WK + (h + 1) * K],
                    mk[h].rearrange("(c i) k -> i c k", i=128),
                )
                nc.sync.dma_start(
                    wstack[:, :, WV + h * V : WV + (h + 1) * V],
                    mv[h].rearrange("(c i) v -> i c v", i=128),
                )
            for p in range(NPAIR):
                for j in range(2):
                    h = 2 * p + j
                    nc.vector.tensor_copy(
                        mqp[p][:, :, j * K : (j + 1) * K],
                        wstack[:, :, WQ + h * K : WQ + (h + 1) * K],
                    )
                    nc.vector.tensor_copy(
                        mkp[p][:, :, j * K : (j + 1) * K],
                        wstack[:, :, WK + h * K : WK + (h + 1) * K],
                    )
            for h in range(HPC):
                nc.vector.tensor_copy(
                    mvp[:, :, h * V : (h + 1) * V],
                    wstack[:, :, WV + h * V : WV + (h + 1) * V],
                )

            # ------------- phase 1: transpose x via PE -------------
            # x loads go to one persistent fp32 buffer, parallel DMAs into
            # disjoint slices (no slot reuse -> single-wait DMAs). Each 128-row
            # tile is cast to fp16, then PE-transposed in fp16.
            for u in range(XS if T("dma_first", 0) else 0, NST, XS):
                nc.sync.dma_start(xbig[:, u : u + XS, :], xr[:, u : u + XS, :])
            TPK = T("tpack", 1)  # transposes packed per psum tile/eviction

            def emit_tr(st):
                xcs = xstage.tile([128, I], F16, tag="xcs", name=f"xcs{rep}_{st}")
                ev_copy(xcs[:], xbig[:, st, :])
                for ci0 in range(0, NCI, TPK):
                    pt = ptr.tile([128, TPK, 128], F16, tag="pt", name=f"pt{rep}_{st}_{ci0}")
                    for j in range(TPK):
                        ci = ci0 + j
                        nc.tensor.transpose(
                            pt[:, j, :], xcs[:, ci * 128 : (ci + 1) * 128], ident[:]
                        )
                    nc.vector.tensor_copy(
                        xT[:, ci0 : ci0 + TPK, st * 128 : (st + 1) * 128], pt[:]
                    )

            if not T("fine2", 0):
                for st in range(NST):
                    emit_tr(st)

            if not T("fine2", 0):
                ptr_ctx.close()   # frees the transpose PSUM bank for phase 3
            # ------------- phase 2: projections -------------
            def emit_qk1(p, sg):
                for sg in [sg]:
                    psq = pproj.tile([128, 512], F32, tag=ptag("psq"), name=f"psq{rep}_{p}_{sg}")
                    psk = pproj.tile([128, 512], F32, tag=ptag("psk"), name=f"psk{rep}_{p}_{sg}")
                    for ci in range(NCI):
                        nc.tensor.matmul(
                            psq[:],
                            lhsT=mqp[p][:, ci, :],
                            rhs=xT[:, ci, sg * 512 : (sg + 1) * 512],
                            start=(ci == 0),
                            stop=(ci == NCI - 1),
                        )
                        nc.tensor.matmul(
                            psk[:],
                            lhsT=mkp[p][:, ci, :],
                            rhs=xT[:, ci, sg * 512 : (sg + 1) * 512],
                            start=(ci == 0),
                            stop=(ci == NCI - 1),
                        )
                    ev_copy(qt[p][:, sg * 512 : (sg + 1) * 512], psq[:])
                    ev_copy(kt[p][:, sg * 512 : (sg + 1) * 512], psk[:])

            def emit_qk(p):
                for sg in range(NSG):
                    emit_qk1(p, sg)

            def emit_v_proj():
                for tt in range(NST):
                    psv = pproj.tile([128, HPC * V], F32, tag=ptag("psv"), name=f"psv{rep}_{tt}")
                    for ci in range(NCI):
                        nc.tensor.matmul(
                            psv[:],
                            lhsT=xT[:, ci, tt * 128 : (tt + 1) * 128],
                            rhs=mvp[:, ci, :],
                            start=(ci == 0),
                            stop=(ci == NCI - 1),
                        )
                    for h in range(HPC):
                        ev_copy(
                            vsb[h][:, tt, 0:V], psv[:, h * V : (h + 1) * V]
                        )

        # ------------- phase 3: attention -------------
        if not FUSE:
            emit_qk(0)
            if NPAIR > 1:
                emit_qk(1)
            emit_v_proj()   # pproj pool closes below in this mode
        stage_ctx.close()
        if not FUSE:
            psum_ctx.close()
        att_ctx = ExitStack()
        expp = att_ctx.enter_context(tc.tile_pool(name=f"expp{rep}", bufs=T("expp", 2)))
        outp = att_ctx.enter_context(tc.tile_pool(name=f"outp{rep}", bufs=T("outp", 4)))
        recp = att_ctx.enter_context(tc.tile_pool(name=f"recp{rep}", bufs=T("recp", 4)))
        psc = att_ctx.enter_context(
            tc.tile_pool(name=f"psc{rep}", bufs=T("psc", 2), space="PSUM")
        )
        pav = att_ctx.enter_context(
            tc.tile_pool(name=f"pav{rep}", bufs=T("pav", 2 if FUSE else 4), space="PSUM")
        )
        if True:
            ECH = T("ech", 1)  # chunks per ACT exp op

            def emit_scores_chunk(p, sg, ex, c):
                ps = psc.tile([128, 1024], F32, tag="ps", name=f"ps{rep}_{p}_{sg}_{c}")
                for j in range(2):
                    nc.tensor.matmul(
                        ps[:, j * 512 : (j + 1) * 512],
                        lhsT=kt[p][j * 64 : (j + 1) * 64, c * 128 : (c + 1) * 128],
                        rhs=qt[p][j * 64 : (j + 1) * 64, sg * 512 : (sg + 1) * 512],
                        start=True,
                        stop=True,
                        tile_position=(j * 64, 0),
                    )
                nc.scalar.activation(
                    ex[:, c, :], ps[:], mybir.ActivationFunctionType.Exp, scale=SCALE,
                )

            def emit_av_sub(p, sg, ex, j, stl):
                h = 2 * p + j
                po = pav.tile([128, V + 1], F32, tag="po", name=f"po{rep}_{p}_{sg}_{j}_{stl}")
                soff = j * 512 + stl * 128
                for c in range(NST):
                    nc.tensor.matmul(
                        po[:],
                        lhsT=ex[:, c, soff : soff + 128],
                        rhs=vsb[h][:, c, 0 : V + 1],
                        start=(c == 0),
                        stop=(c == NST - 1),
                    )
                rec = recp.tile([128, 1], F32, tag="rec", name=f"rec{rep}_{p}_{sg}_{j}_{stl}")
                nc.vector.reciprocal(rec[:], po[:, V : V + 1])
                ob = outp.tile([128, V], F32, tag="ob", name=f"ob{rep}_{p}_{sg}_{j}_{stl}")
                nc.vector.tensor_scalar_mul(ob[:], po[:, 0:V], rec[:])
                row0 = sg * 512 + stl * 128
                nc.sync.dma_start(out[h, row0 : row0 + 128, :], ob[:])

            def emit_scores_exp(p, sg):
                # scoresT + exp for both heads of the pair; returns the expT tile
                ex = expp.tile([128, NST, 1024], F16, tag="ex", name=f"ex{rep}_{p}_{sg}")
                for c0 in range(0, NST, ECH):
                    ps = psc.tile([128, ECH, 1024], F32, tag="ps", name=f"ps{rep}_{p}_{sg}_{c0}")
                    for cj in range(ECH):
                        c = c0 + cj
                        for j in range(2):
                            nc.tensor.matmul(
                                ps[:, cj, j * 512 : (j + 1) * 512],
                                lhsT=kt[p][j * 64 : (j + 1) * 64, c * 128 : (c + 1) * 128],
                                rhs=qt[p][j * 64 : (j + 1) * 64, sg * 512 : (sg + 1) * 512],
                                start=True,
                                stop=True,
                                tile_position=(j * 64, 0),
                            )
                    nc.scalar.activation(
                        ex[:, c0 : c0 + ECH, :], ps[:],
                        mybir.ActivationFunctionType.Exp,
                        scale=SCALE,
                    )
                return ex

            def emit_av(p, sg, ex):
                # AV + fused softmax denominator (ones column of vsb)
                for j in range(2):
                    h = 2 * p + j
                    for stl in range(4):
                        po = pav.tile([128, V + 1], F32, tag="po", name=f"po{rep}_{p}_{sg}_{j}_{stl}")
                        soff = j * 512 + stl * 128
                        for c in range(NST):
                            nc.tensor.matmul(
                                po[:],
                                lhsT=ex[:, c, soff : soff + 128],
                                rhs=vsb[h][:, c, 0 : V + 1],
                                start=(c == 0),
                                stop=(c == NST - 1),
                            )
                        rec = recp.tile([128, 1], F32, tag="rec", name=f"rec{rep}_{p}_{sg}_{j}_{stl}")
                        nc.vector.reciprocal(rec[:], po[:, V : V + 1])
                        ob = outp.tile([128, V], F32, tag="ob", name=f"ob{rep}_{p}_{sg}_{j}_{stl}")
                        nc.vector.tensor_scalar_mul(ob[:], po[:, 0:V], rec[:])
                        row0 = sg * 512 + stl * 128
                        nc.sync.dma_start(out[h, row0 : row0 + 128, :], ob[:])

            seq = [(p, sg) for p in range(NPAIR) for sg in range(NSG)]
            AHEAD = T("ahead", 0)
            FINE = T("fine", 0)
            if T("fine2", 0):
                assert FUSE, "fine2 requires fuse"
                # progressive start: per 4-st group, transpose -> qk(p0,g) ->
                # first exp group's score chunks 4g..4g+3. ACT starts as soon
                # as the first 4 transposed tiles and one qk group exist.
                ex_tiles = {}
                ex0 = expp.tile([128, NST, 1024], F16, tag="ex", name=f"ex{rep}_0_0")
                ex_tiles[seq[0]] = ex0
                ptr_open = True
                for g in range(NSG):
                    for st in range(4 * g, 4 * g + 4):
                        emit_tr(st)
                    emit_qk1(0, g)
                for g in range(NSG):
                    if g == 0:
                        ptr_ctx.close()  # free transpose bank before psc allocs
                    for c in range(4 * g, 4 * g + 4):
                        emit_scores_chunk(0, 0, ex0, c)
                    if NPAIR > 1:
                        emit_qk1(1, g)
                # prefill remaining ahead groups (AHEAD=2 recommended: group
                # k+AHEAD must not reuse the SBUF slot group k is draining)
                for k in range(1, min(AHEAD, len(seq))):
                    ex_tiles[seq[k]] = emit_scores_exp(*seq[k])
                emit_v_proj()
                subs = [(j, stl) for j in range(2) for stl in range(4)]
                for k, (p, sg) in enumerate(seq):
                    ex = ex_tiles.pop((p, sg))
                    nk = seq[k + AHEAD] if k + AHEAD < len(seq) else None
                    if nk is not None:
                        nex = expp.tile([128, NST, 1024], F16, tag="ex",
                                        name=f"ex{rep}_{nk[0]}_{nk[1]}")
                        ex_tiles[nk] = nex
                        for c in range(NST):
                            emit_scores_chunk(nk[0], nk[1], nex, c)
                            if c % 2 == 1:
                                emit_av_sub(p, sg, ex, *subs[c // 2])
                    else:
                        for sb in subs:
                            emit_av_sub(p, sg, ex, *sb)
            elif FINE:
                assert FUSE and AHEAD, "fine requires fuse+ahead"
                # qk(p0) first so the exp stream starts as early as possible,
                # then pre-fill AHEAD exp groups (qk(p1) interleaved), then V.
                emit_qk(0)
                ex_tiles = {}
                ex_tiles[seq[0]] = emit_scores_exp(*seq[0])
                if NPAIR > 1:
                    emit_qk(1)
                for k in range(1, min(AHEAD, len(seq))):
                    ex_tiles[seq[k]] = emit_scores_exp(*seq[k])
                emit_v_proj()
                # steady state: AV sub-blocks of group k interlaced with score
                # chunks of group k+AHEAD at ~2-chunk granularity so the ACT
                # exp stream never runs dry behind monolithic AV blocks.
                subs = [(j, stl) for j in range(2) for stl in range(4)]
                for k, (p, sg) in enumerate(seq):
                    ex = ex_tiles.pop((p, sg))
                    nk = seq[k + AHEAD] if k + AHEAD < len(seq) else None
                    if nk is not None:
                        nex = expp.tile([128, NST, 1024], F16, tag="ex",
                                        name=f"ex{rep}_{nk[0]}_{nk[1]}")
                        ex_tiles[nk] = nex
                        for c in range(NST):
                            emit_scores_chunk(nk[0], nk[1], nex, c)
                            if c % 2 == 1:
                                emit_av_sub(p, sg, ex, *subs[c // 2])
                    else:
                        for sb in subs:
                            emit_av_sub(p, sg, ex, *sb)
            elif AHEAD:
                assert FUSE, "ahead requires fuse (pools must coexist)"
                emit_qk(0)
                if NPAIR > 1:
                    emit_qk(1)
                ex_tiles = {}
                for k in range(min(AHEAD, len(seq))):
                    ex_tiles[seq[k]] = emit_scores_exp(*seq[k])
                emit_v_proj()
                SWAP = T("swap", 0)  # scores(k+AHEAD) before AV(k): keeps the
                # ACT exp stream fed at cycle start (requires AHEAD < expp
                # bufs so the slot being written isn't the one AV(k) reads)
                for k, (p, sg) in enumerate(seq):
                    if SWAP and k + AHEAD < len(seq):
                        ex_tiles[seq[k + AHEAD]] = emit_scores_exp(*seq[k + AHEAD])
                    emit_av(p, sg, ex_tiles.pop((p, sg)))
                    if not SWAP and k + AHEAD < len(seq):
                        ex_tiles[seq[k + AHEAD]] = emit_scores_exp(*seq[k + AHEAD])
            else:
                if FUSE:
                    emit_qk(0)
                    if NPAIR > 1:
                        emit_qk(1)
                    emit_v_proj()
                for p, sg in seq:
                    ex = emit_scores_exp(p, sg)
                    emit_av(p, sg, ex)
        att_ctx.close()
        if FUSE:
            psum_ctx.close()


_NC_CACHE = {}

# Best-measured tuning (TimelineSim sweep): pack 8 transposes per PSUM
# tile/eviction; fused PSUM pools + 3-group exp-ahead software pipeline so
# ACT exp (the phase-3 bottleneck) starts during the projection phase;
# per-tile x DMAs for an earlier pipeline start.
DEFAULT_TUNE = {"tpack": 8, "fuse": 1, "ahead": 3, "expp": 3, "xsplit": 1}


def _install_neff_cache():
    """Persistent on-disk NEFF cache keyed on BIR hash. Saves the ~15min
    neuronxcc compile on repeat runs of the same program on this machine."""
    try:
        import hashlib
        import os
        import shutil

        import concourse.bass_utils as bu
        from concourse import bass2jax

        if getattr(bu.compile_bir_kernel, "_is_cached_wrapper", False):
            return
        orig = bu.compile_bir_kernel
        cache_dir = "/root/neffcache"

        def cached(bir_json, tmpdir, neff_name="file.neff"):
            try:
                h = hashlib.sha256(bir_json).hexdigest()[:24]
                cpath = os.path.join(cache_dir, f"{h}.neff")
                if os.path.exists(cpath):
                    dst = os.path.join(tmpdir, neff_name)
                    shutil.copy(cpath, dst)
                    return dst
                p = orig(bir_json, tmpdir, neff_name)
                os.makedirs(cache_dir, exist_ok=True)
                shutil.copy(p, cpath)
                return p
            except OSError:
                return orig(bir_json, tmpdir, neff_name)

        cached._is_cached_wrapper = True
        bu.compile_bir_kernel = cached
        bass2jax.compile_bir_kernel = cached
    except Exception:
        pass


def _get_nc():
    if "nc" not in _NC_CACHE:
        _NC_CACHE["nc"] = build_attention_nc(tune=DEFAULT_TUNE)
    return _NC_CACHE["nc"]


def run_sharded(x, Mq, Mk, Mv, **spmd_kwargs):
    """Shard inputs over 8 cores, run, reassemble. Returns (out, BassKernelResults)."""
    _install_neff_cache()
    from concourse.bass_utils import run_bass_kernel_spmd

    B, S, I = x.shape
    H = Mq.shape[0]
    V = Mv.shape[-1]
    HPC = H // 2  # 4 heads per core, 2 head groups
    x = np.asarray(x, dtype=np.float32)
    Mq = np.asarray(Mq, dtype=np.float32)
    Mk = np.asarray(Mk, dtype=np.float32)
    Mv = np.asarray(Mv, dtype=np.float32)

    in_maps = []
    for c in range(8):
        b, hg = c // 2, c % 2
        hs = slice(hg * HPC, (hg + 1) * HPC)
        in_maps.append(
            {
                "x": np.ascontiguousarray(x[b]),
                "mq": np.ascontiguousarray(Mq[hs, 0]),
                "mk": np.ascontiguousarray(Mk[hs, 0]),
                "mv": np.ascontiguousarray(Mv[hs, 0]),
            }
        )

    nc = _get_nc()
    br = run_bass_kernel_spmd(nc, in_maps, list(range(8)), **spmd_kwargs)

    outf = np.empty((H, B, S, V), dtype=np.float32)
    for c in range(8):
        b, hg = c // 2, c % 2
        outf[hg * HPC : (hg + 1) * HPC, b] = br.results[c]["out"]
    return outf, br


def kernel(x, Mq, Mk, Mv):
    """Full inputs -> full output (H, B, S, V). Shards over 8 NeuronCores."""
    out, _ = run_sharded(x, Mq, Mk, Mv)
    return out
